# revision 4
# baseline (speedup 1.0000x reference)
"""MeshGNN Trainium2 kernel v2 (fp8 DoubleRow matmuls, 2-engine PSUM relu).

Same math as the baseline (network collapses to a per-row MLP; see
_fold_weights), restructured for engine occupancy:

  - hidden PSUM ring is 3 x [128,2,512] tiles (6 banks) so matmul refill
    latency stays off the relu critical path, with 2 banks left over for a
    dedicated out-stage pool (the baseline's single 4-tile ring made the
    l3/out stage allocations perturb the hidden pipeline).
  - out stage uses its own 2-bank PSUM tile (matmul dst partition base must
    be 0 on TRN2, so the two blocks of a pair sit in separate banks), one
    copy + one store per pair.
  - relu engine schedule (ACT vs DVE) and copy placement are tunable and
    were picked by cost-model (TimelineSim) search.
"""

import numpy as np

# ---------------------------------------------------------------- constants
B = 32768
CORES = 8
ROWS = B // CORES            # 4096 rows per core
TD = 384                     # text dim
KPAD = 512                   # L0 contraction padded to 4 k-tiles
H = 256                      # hidden
OUT = 36                     # 12 verts * 3 coords
NBLK = 8                     # row blocks per core
N = ROWS // NBLK             # 512 rows per block
NW = 22                      # packed weight chunks of [128, 144]
PBASE = 64                   # partition base of the odd block in out PSUM

# relu engine schedule [layer][block]: A=ACT, D=DVE.
# GPSIMD cannot read PSUM on TRN2, so only these two engines can drain it.
RELU_ENG = (
    "ADADADAD",
    "ADADADAD",
    "ADADADAD",
    "ADADADAA",
)
COPY_ENG = "DDADDADA"        # out-copy engine per block (block_out mode)

_BUILT = {}                  # cache: compiled Bass modules keyed by config


def _fp8_np():
    import concourse.mybir as mybir
    return mybir.dt.np(mybir.dt.float8e4)


def _build_bass(repeat=1, loop_repeat=0, zero_bias=None,
                relu_eng=None, copy_eng=None, pbase=None,
                op_bufs=4, split_last=False, prologue=True, block_out=True):
    """Build + compile the per-core Bass program (same NEFF on all cores).

    loop_repeat > 0 wraps the pipeline in a device-side For_i loop executed
    that many times (identical outputs) -- used for timing with enough
    device work to swamp dispatch noise entirely.  The loop uses
    staggered semaphore reset: no all-engine rendezvous at the back-edge,
    but iterations remain effectively serialized by their data
    dependencies (PE executes in order, and the PSUM/h-tile rings make
    iteration i+1's first matmuls wait on iteration i's tail), so each
    iteration is still a full fill+drain; measured per-iteration time
    matches the unrolled two-reps-per-body marginal to within ~2us of
    bookkeeping.
    """
    import contextlib

    import concourse.mybir as mybir
    import concourse.tile as tile
    from concourse import bacc

    if zero_bias is None:
        zero_bias = _BUILT.get("zero_bias", True)
    relu_eng = relu_eng or RELU_ENG
    copy_eng = copy_eng or COPY_ENG
    pbase = PBASE if pbase is None else pbase

    f32 = mybir.dt.float32
    fp8 = mybir.dt.float8e4
    bf16 = mybir.dt.bfloat16
    DR = mybir.MatmulPerfMode.DoubleRow
    RELU = mybir.ActivationFunctionType.Relu
    IDENT = mybir.ActivationFunctionType.Identity
    ADD = mybir.AluOpType.add
    MAX = mybir.AluOpType.max

    nc = bacc.Bacc(
        "TRN2",
        target_bir_lowering=False,
        debug=False,
        enable_asserts=False,
        num_devices=CORES,
    )

    # x block-major: row p holds, per block b, the 4 k-tiles' 512 columns
    xt_d = nc.dram_tensor("xt", (128, NBLK * 4 * N), fp8, kind="ExternalInput")
    w_d = nc.dram_tensor("wpk", (128, NW * 144), fp8, kind="ExternalInput")
    bl_d = None if zero_bias else [
        nc.dram_tensor(f"b{l}", (128, 2), f32, kind="ExternalInput")
        for l in range(4)
    ]
    out_d = nc.dram_tensor("out", (OUT, ROWS), bf16, kind="ExternalOutput")

    xt_v = xt_d.ap().rearrange("p (b k n) -> p b k n", k=4, n=N)
    out_v = out_d.ap().rearrange("p (b n) -> p b n", n=N)

    # packed-weight chunk index for each DR stationary operand
    ch_l0 = lambda m, pair: 4 * m + 2 * pair          # noqa: E731
    ch_l = lambda l, m: 8 + 4 * (l - 1) + 2 * m       # noqa: E731
    CH_L4 = 20

    with tile.TileContext(nc) as tc:
        with (
            tc.tile_pool(name="wp", bufs=1) as wp,
            tc.tile_pool(name="xp", bufs=1) as xp,
            tc.tile_pool(name="hp", bufs=2) as hp,
            tc.tile_pool(name="op", bufs=op_bufs) as op,
            tc.tile_pool(name="pp", bufs=3, space="PSUM") as pp,
            tc.tile_pool(
                name="qp", bufs=2 if block_out else 1, space="PSUM"
            ) as qp,
        ):
            # ---- weights / biases: one packed image, loaded once
            wsb = wp.tile([128, NW, 144], fp8, tag="w")
            if prologue:
                nc.sync.dma_start(
                    wsb[:, :, :],
                    w_d.ap().rearrange("p (a b) -> p a b", b=144),
                )
            blt = {}
            if not zero_bias:
                for l in range(4):
                    t = wp.tile([128, 2], f32, tag=f"b{l}")
                    if prologue:
                        nc.sync.dma_start(t[:], bl_d[l].ap()[:])
                    blt[l] = t

            xt = xp.tile([128, NBLK, 4, N], fp8, tag="x")

            # dummy 1-elem activation before the loop: forces the Relu/Ident
            # ACT table load to happen once at startup, not inside For_i
            warm = wp.tile([1, 1], f32, tag="warm")
            nc.gpsimd.memset(warm[:], 0)
            nc.scalar.activation(warm[:], warm[:], RELU)

            if prologue:
                # prologue x load in consumption order (weights DMA above
                # goes first on the sync queue; x chunks stream in behind
                # it, split across ACT and SP so descriptor generation
                # overlaps).  prologue=False models a steady-state For_i
                # iteration, where x/weights are already resident.
                nc.scalar.dma_start(xt[:, 0:2, :, :], xt_v[:, 0:2, :, :])
                nc.sync.dma_start(xt[:, 2:4, :, :], xt_v[:, 2:4, :, :])
                nc.scalar.dma_start(xt[:, 4:6, :, :], xt_v[:, 4:6, :, :])
                nc.sync.dma_start(xt[:, 6:8, :, :], xt_v[:, 6:8, :, :])
            else:
                # sim-only stand-in writes so the tile framework sees the
                # resident tensors as initialized
                nc.gpsimd.memset(wsb[0:1, 0:1, 0:1], 0)
                nc.gpsimd.memset(xt[0:1, 0:1, 0:1, 0:1], 0)

            def relu(l, b, dst, src):
                e = relu_eng[l][b]
                if e == "A":
                    nc.scalar.activation(dst, src, RELU)
                elif e == "S":      # split halves across both engines
                    nc.scalar.activation(dst[:, 0, :], src[:, 0, :], RELU)
                    nc.vector.tensor_scalar(
                        dst[:, 1, :], src[:, 1, :], 0.0, None, MAX
                    )
                else:
                    nc.vector.tensor_scalar(dst, src, 0.0, None, MAX)

            def out_copy(eng, dst, src):
                if eng == "A":
                    nc.scalar.activation(dst, src, IDENT)
                else:
                    nc.vector.tensor_scalar(dst, src, 0.0, None, ADD)

            def relu_bias(l, b, m, dst, src, bias_ap):
                if relu_eng[l][b] == "A":
                    nc.scalar.activation(dst, src, RELU, bias=bias_ap)
                else:
                    nc.vector.tensor_scalar(dst, src, bias_ap, 0.0, ADD, MAX)

            loop_cm = (
                tc.For_i(0, loop_repeat, 1, staggered_reset=True)
                if loop_repeat else contextlib.nullcontext()
            )
            with loop_cm:
                for rep in range(repeat):
                    # store queue per pair; the last pair stays on the
                    # fast SP HWDGE path so the final store chain is short
                    STORE_Q = (nc.sync, nc.gpsimd, nc.sync, nc.sync)

                    ob_cur = [None]

                    def emit_block_out(b):
                        """L4 matmul for one block into a 1-bank out tile
                        (matmul dst partition base must be 0 on TRN2), a
                        free-size-512 copy into half of the pair's staging
                        tile, and one store per completed pair."""
                        q = qp.tile([128, 1, N], f32, tag="q")
                        nc.tensor.matmul(
                            q[0:OUT, 0, :],
                            wsb[:, CH_L4:CH_L4 + 2, 0:OUT],
                            h_prev[b][:, 0:2, :],
                            start=True, stop=True, perf_mode=DR,
                        )
                        if b % 2 == 0:
                            ob_cur[0] = op.tile(
                                [OUT, 2, N], bf16, name=f"obp{b}", tag="ob"
                            )
                        ob = ob_cur[0]
                        out_copy(
                            copy_eng[b], ob[:, b % 2, :], q[0:OUT, 0, :]
                        )
                        if b % 2 == 1:
                            STORE_Q[b // 2].dma_start(
                                out_v[:, b - 1:b + 1, :], ob[:, :, :]
                            )

                    def emit_pair(pr):
                        """L4 matmuls for blocks (2pr, 2pr+1) into the
                        dedicated 2-bank out tile, one copy to SBUF, one
                        store (hardware requires matmul dst partition base
                        0, so blocks sit in separate banks)."""
                        be, bo = 2 * pr, 2 * pr + 1
                        q = qp.tile([128, 2, N], f32, tag="q")
                        nc.tensor.matmul(
                            q[0:OUT, 0, :],
                            wsb[:, CH_L4:CH_L4 + 2, 0:OUT],
                            h_prev[be][:, 0:2, :],
                            start=True, stop=True, perf_mode=DR,
                        )
                        nc.tensor.matmul(
                            q[0:OUT, 1, :],
                            wsb[:, CH_L4:CH_L4 + 2, 0:OUT],
                            h_prev[bo][:, 0:2, :],
                            start=True, stop=True, perf_mode=DR,
                        )
                        ob = op.tile([OUT, 2, N], bf16, tag="ob")
                        out_copy(copy_eng[pr], ob[:, :, :], q[0:OUT, :, :])
                        STORE_Q[pr].dma_start(
                            out_v[:, be:be + 2, :], ob[:, :, :]
                        )

                    h_prev = {}
                    for l in range(4):
                        for b in range(NBLK):
                            ps = pp.tile([128, 2, N], f32, tag="ps")
                            h = hp.tile(
                                [128, 2, N], fp8,
                                name=f"h{l}{b}", tag=f"h{b}",
                            )
                            if l == 0:
                                for m in range(2):
                                    c0, c1 = ch_l0(m, 0), ch_l0(m, 1)
                                    nc.tensor.matmul(
                                        ps[:, m, :],
                                        wsb[:, c0:c0 + 2, 0:128],
                                        xt[:, b, 0:2, :],
                                        start=True, stop=False,
                                        perf_mode=DR,
                                    )
                                    nc.tensor.matmul(
                                        ps[:, m, :],
                                        wsb[:, c1:c1 + 2, 0:128],
                                        xt[:, b, 2:4, :],
                                        start=False, stop=True,
                                        perf_mode=DR,
                                    )
                            else:
                                for m in range(2):
                                    c = ch_l(l, m)
                                    nc.tensor.matmul(
                                        ps[:, m, :],
                                        wsb[:, c:c + 2, 0:128],
                                        h_prev[b][:, 0:2, :],
                                        start=True, stop=True,
                                        perf_mode=DR,
                                    )
                            if zero_bias:
                                if split_last and l == 3 and b == NBLK - 1:
                                    # final block: relu halves on both engines
                                    # in parallel to shorten the drain chain
                                    nc.scalar.activation(
                                        h[:, 0, :], ps[:, 0, :], RELU
                                    )
                                    nc.vector.tensor_scalar(
                                        h[:, 1, :], ps[:, 1, :], 0.0, None, MAX
                                    )
                                else:
                                    relu(l, b, h[:, :, :], ps[:, :, :])
                            else:
                                for m in range(2):
                                    relu_bias(
                                        l, b, m, h[:, m, :], ps[:, m, :],
                                        blt[l][:, m:m + 1],
                                    )
                            h_prev[b] = h
                            if l == 3 and block_out:
                                emit_block_out(b)
                            elif l == 3 and b % 2 == 1:
                                emit_pair(b // 2)

                        if l == 0:
                            # layer 0 has read all of xt: refill it for the
                            # next iteration, overlapped with layers 1-3
                            for lo, hi in ((0, 2), (2, 4), (4, 6), (6, 8)):
                                nc.sync.dma_start(
                                    xt[:, lo:hi, :, :],
                                    xt_v[:, lo:hi, :, :],
                                )

    nc.compile()
    return nc


def _fold_weights(W_text, b_text, W_gnn, b_gnn, W_out, b_out, adjacency, template):
    s_rows = adjacency.astype(np.float64).sum(axis=1)
    if np.ptp(s_rows) > 1e-5:
        raise ValueError("adjacency row sums are not uniform; collapse invalid")
    s = float(s_rows.mean())

    W0c = (W_text.astype(np.float64) @ (s * W_gnn[0].astype(np.float64)))
    b0c = s * (b_text.astype(np.float64) @ W_gnn[0].astype(np.float64)) + b_gnn[0]
    Wl = [s * W_gnn[l].astype(np.float64) for l in (1, 2, 3)]
    bl = [b_gnn[l] for l in (1, 2, 3)]
    W4c = np.tile(W_out, (1, 12))
    b4c = np.tile(b_out, 12) + template.reshape(OUT)
    biases = [np.asarray(b, dtype=np.float32) for b in [b0c, *bl]]
    return W0c, Wl, W4c, biases, np.asarray(b4c, dtype=np.float32)


def _pack_weights(W0c, Wl, W4c):
    """Pack all matmul weights into the [128, NW, 144] fp8 SBUF image.

    Chunk pairs (c, c+1) hold a DR stationary operand: element (p, i, m) of
    view [:, c:c+2, 0:M] must equal W[pair_k0*128 + i*128 + p, m]."""
    fp8 = _fp8_np()
    img = np.zeros((128, NW, 144), dtype=fp8)

    def put(c, Wsub):                      # Wsub: (256, M) fp8
        M = Wsub.shape[1]
        img[:, c, :M] = Wsub[0:128]
        img[:, c + 1, :M] = Wsub[128:256]

    W0p = np.zeros((KPAD, H), dtype=fp8)
    W0p[0:TD] = W0c.astype(np.float32).astype(fp8)
    Wlq = [w.astype(np.float32).astype(fp8) for w in Wl]
    W4q = W4c.astype(np.float32).astype(fp8)

    for m in range(2):
        ms = slice(m * 128, (m + 1) * 128)
        put(4 * m + 0, W0p[0:256, ms])
        put(4 * m + 2, W0p[256:512, ms])
    for li in range(3):
        for m in range(2):
            put(8 + 4 * li + 2 * m, Wlq[li][:, m * 128:(m + 1) * 128])
    put(20, W4q)
    return np.ascontiguousarray(img.reshape(128, NW * 144))


def _make_in_maps(inputs):
    x = np.asarray(inputs["text_emb"], dtype=np.float32)
    W0c, Wl, W4c, biases, b4c = _fold_weights(
        np.asarray(inputs["W_text"]), np.asarray(inputs["b_text"]),
        np.asarray(inputs["W_gnn"]), np.asarray(inputs["b_gnn"]),
        np.asarray(inputs["W_out"]), np.asarray(inputs["b_out"]),
        np.asarray(inputs["adjacency"]), np.asarray(inputs["template"]),
    )
    zero_bias = all(np.all(b == 0.0) for b in biases)
    _BUILT.setdefault("zero_bias", zero_bias)
    _BUILT["b4c"] = b4c
    fp8 = _fp8_np()
    wimg = _pack_weights(W0c, Wl, W4c)
    in_maps = []
    for c in range(CORES):
        xpad = np.zeros((KPAD, ROWS), dtype=fp8)
        xpad[0:TD] = np.ascontiguousarray(
            x[c * ROWS:(c + 1) * ROWS].T
        ).astype(fp8)
        # block-major pack: (p, b, k, j) = xpad[k*128 + p, b*N + j]
        xb = np.ascontiguousarray(
            xpad.reshape(4, 128, NBLK, N).transpose(1, 2, 0, 3)
        ).reshape(128, NBLK * 4 * N)
        m = {"xt": xb, "wpk": wimg}
        if not _BUILT["zero_bias"]:
            for l in range(4):
                m[f"b{l}"] = np.ascontiguousarray(
                    biases[l].reshape(2, 128).T.astype(np.float32)
                )
        in_maps.append(m)
    return in_maps


def kernel(**inputs):
    from concourse.bass_utils import run_bass_kernel_spmd

    in_maps = _make_in_maps(inputs)
    if "nc" not in _BUILT:
        _BUILT["nc"] = _build_bass(repeat=1)
    nc = _BUILT["nc"]
    res = run_bass_kernel_spmd(nc, in_maps, core_ids=list(range(CORES)))
    _BUILT["last_results"] = res
    _BUILT["last_in_maps"] = in_maps

    b4c = _BUILT["b4c"]
    full = np.empty((B, OUT), dtype=np.float32)
    for c in range(CORES):
        o = np.asarray(
            res.results[c]["out"], dtype=np.float32
        ).reshape(OUT, ROWS)
        full[c * ROWS:(c + 1) * ROWS] = o.T
    full += b4c[None, :]
    return full.reshape(B, 12, 3)
